# revision 1
# baseline (speedup 1.0000x reference)
"""Distributed Trainium2 kernel for nn_Attention_31104153157828.

Computation (B=16, S=2048, D=1024):
    fac1 = k @ W                     [B,S,D]
    fac2 = (q @ U)[:, None, :]       [B,1,D]
    t    = tanh(fac1 + fac2)
    s    = einsum('bsd,bse->bde', v, t)      [B,D,D]
    attn = softmax(s, axis=0)                 (softmax over BATCH)
    out  = einsum('bsd,bde->bse', v, attn)   [B,S,D]

Sharding: data-parallel over batch, 2 batches per core on 8 cores.
The batch-axis softmax needs cross-core AllReduce of max and sum(exp)
over the [D,D] logit matrix, done in 4 chunks.

Per-core dataflow (b0, b1 local batches):
  Stage A (per b): k^T via PE transpose; fac1 = k^T.T @ W in fp32r,
      fac2 folded in via a K=1 matmul; t = tanh(.) in fp32r.
  Stage B (per b): s[d,e] = v^T t via matmul with v natural [s,d] as
      stationary and t [s,e] as moving, fp32r, accumulated over s in
      PSUM per e-half (8 banks).
  Softmax: chunk = 2 d-tiles: local max -> AllReduce(max) -> subtract,
      exp -> local sum -> AllReduce(add) -> reciprocal -> attn (bf16).
  Stage C (per b): out = v @ attn with v^T in bf16 via xbar DMA
      transpose as stationary, attn bf16 moving, fp32 PSUM accumulate.
"""
import numpy as np
import concourse.bass as bass
import concourse.bacc as bacc
import concourse.tile as tile
import concourse.mybir as mybir
from concourse.bass_utils import run_bass_kernel_spmd

F32 = mybir.dt.float32
F32R = mybir.dt.float32r
BF16 = mybir.dt.bfloat16
AF = mybir.ActivationFunctionType

B, S, D = 16, 2048, 1024
N_CORES = 8
BL = B // N_CORES          # local batches per core = 2
M_T = S // 128             # 16 s-tiles
KC = D // 128              # 8 contraction chunks (d)
EH = 2                     # e halves of 512
ARC = 4                    # AllReduce chunks (pairs of d-tiles)
RG = [list(range(N_CORES))]


def build():
    nc = bacc.Bacc("TRN2", target_bir_lowering=False, debug=False,
                   num_devices=N_CORES)

    q2 = nc.dram_tensor("q2", [BL, D], F32, kind="ExternalInput")
    k2 = nc.dram_tensor("k2", [BL, S, D], F32, kind="ExternalInput")
    v2 = nc.dram_tensor("v2", [BL, S, D], F32, kind="ExternalInput")
    Wd = nc.dram_tensor("W", [D, D], F32, kind="ExternalInput")
    Ud = nc.dram_tensor("U", [D, D], F32, kind="ExternalInput")
    out2 = nc.dram_tensor("out", [BL, S, D], F32, kind="ExternalOutput")

    # collective bounce buffers, one set per e-half
    mx_in = [nc.dram_tensor(f"mx_in{h}", [128, KC, 512], BF16) for h in range(EH)]
    mx_out = [nc.dram_tensor(f"mx_out{h}", [128, KC, 512], BF16) for h in range(EH)]
    sm_in = [nc.dram_tensor(f"sm_in{h}", [128, KC, 512], BF16) for h in range(EH)]
    sm_out = [nc.dram_tensor(f"sm_out{h}", [128, KC, 512], BF16) for h in range(EH)]

    warm_in = nc.dram_tensor("warm_in", [128, 16], F32)
    warm_out = nc.dram_tensor("warm_out", [128, 16], F32)
    warm_out2 = nc.dram_tensor("warm_out2", [128, 16], F32)

    ident_d = nc.inline_tensor(np.eye(128, dtype=np.float32), name="ident")
    ones_d = nc.inline_tensor(np.ones((1, 128), np.float32), name="ones1")

    with tile.TileContext(nc) as tc:
        with tc.tile_pool(name="rp", bufs=1) as rp:
            ident = rp.tile([128, 128], F32, name="ident_t")
            nc.sync.dma_start(ident[:], ident_d.ap())

            # warm up the collective machinery early (first AR pays ~70us)
            wtile = rp.tile([128, 16], F32, name="wtile")
            nc.gpsimd.dma_start(wtile[:], ident_d.ap()[:, 0:16])
            nc.gpsimd.dma_start(warm_in.ap(), wtile[:])
            ar_w1 = nc.gpsimd.collective_compute(
                "AllReduce", mybir.AluOpType.max, replica_groups=RG,
                ins=[warm_in.ap().opt()], outs=[warm_out.ap().opt()])
            ar_w2 = nc.gpsimd.collective_compute(
                "AllReduce", mybir.AluOpType.add, replica_groups=RG,
                ins=[warm_out.ap().opt()], outs=[warm_out2.ap().opt()])

            # ---- s logits, per local batch [128, KC, D] f32 ----
            sp_cm = tc.tile_pool(name="spool", bufs=1)
            spool = sp_cm.__enter__()
            s_sb = [spool.tile([128, KC, D], F32, name=f"s{b}") for b in range(BL)]

            wp_cm = tc.tile_pool(name="wp", bufs=1)
            wp = wp_cm.__enter__()
            # ---- stage A/B residents: W, fac2, t (freed before softmax) ----
            W_r = wp.tile([128, KC, D], F32R, name="W_r")
            nc.gpsimd.dma_start(W_r[:], Wd.ap().rearrange("(kc p) e -> p kc e", p=128))
            ones_r = wp.tile([1, 128], F32R, name="ones_r")
            nc.gpsimd.dma_start(ones_r[:], ones_d.ap())

            # ---- fac2 = q @ U, per local batch -> [1, BL, D] f32r ----
            fac2 = wp.tile([1, BL, D], F32R, name="fac2")
            with (
                tc.tile_pool(name="f2", bufs=2) as f2p,
                tc.tile_pool(name="f2u", bufs=1) as f2u,
                tc.tile_pool(name="f2ps", bufs=2, space="PSUM") as f2ps,
            ):
                U_r = f2u.tile([128, KC, D], F32R, name="U_r")
                nc.gpsimd.dma_start(
                    U_r[:], Ud.ap().rearrange("(kc p) e -> p kc e", p=128))
                for b in range(BL):
                    qcol = f2p.tile([128, KC], F32R, tag="qcol", name=f"qcol{b}")
                    nc.gpsimd.dma_start(
                        qcol[:], q2.ap()[b].rearrange("(kc p) -> p kc", p=128))
                    for h in range(EH):
                        ps = f2ps.tile([1, 512], F32, tag="f2ps", name=f"f2ps{b}_{h}")
                        for kc in range(KC):
                            nc.tensor.matmul(ps[:], qcol[:, kc:kc + 1],
                                             U_r[:, kc, h * 512:(h + 1) * 512],
                                             start=(kc == 0), stop=(kc == KC - 1))
                        nc.scalar.copy(fac2[0:1, b, h * 512:(h + 1) * 512], ps[:])

            # ======== stages A and B per local batch ========
            for b in range(BL):
                t_b = wp.tile([128, M_T, D], F32R, tag="t", name=f"t{b}")

                # -- stage A: t = tanh(k @ W + fac2) --
                with (
                    tc.tile_pool(name=f"A{b}", bufs=3) as ap_,
                    tc.tile_pool(name=f"A{b}kt", bufs=2) as ktp,
                    tc.tile_pool(name=f"A{b}ps", bufs=3, space="PSUM") as aps,
                    tc.tile_pool(name=f"A{b}tp", bufs=2, space="PSUM") as tps,
                ):
                    for m in range(M_T):
                        kslab = ap_.tile([128, D], F32, tag="kslab",
                                         name=f"kslab{b}_{m}")
                        nc.sync.dma_start(
                            kslab[:], k2.ap()[b, m * 128:(m + 1) * 128, :])
                        kT = ktp.tile([128, KC * 128], F32R, tag="kT",
                                      name=f"kT{b}_{m}")
                        for kc in range(KC):
                            ptr = tps.tile([128, 128], F32, tag="ptr",
                                           name=f"ptr{b}_{m}_{kc}")
                            nc.tensor.transpose(
                                ptr[:], kslab[:, kc * 128:(kc + 1) * 128], ident[:])
                            nc.vector.tensor_copy(kT[:, kc * 128:(kc + 1) * 128],
                                                  ptr[:])
                        for h in range(EH):
                            ps = aps.tile([128, 512], F32, tag="aps",
                                          name=f"aps{b}_{m}_{h}")
                            for kc in range(KC):
                                nc.tensor.matmul(
                                    ps[:], kT[:, kc * 128:(kc + 1) * 128],
                                    W_r[:, kc, h * 512:(h + 1) * 512],
                                    start=(kc == 0), stop=False)
                            nc.tensor.matmul(
                                ps[:], ones_r[:],
                                fac2[0:1, b, h * 512:(h + 1) * 512],
                                start=False, stop=True)
                            nc.scalar.activation(
                                t_b[:, m, h * 512:(h + 1) * 512], ps[:], AF.Tanh)

                # -- stage B: s[d, e] = sum_s v[s, d] * t[s, e] --
                with (
                    tc.tile_pool(name=f"B{b}", bufs=4) as bp,
                    tc.tile_pool(name=f"B{b}ps", bufs=1, space="PSUM") as bps,
                ):
                    for h in range(EH):
                        psb = [bps.tile([128, 512], F32, tag=f"pb{dt}",
                                        name=f"pb{b}_{h}_{dt}") for dt in range(KC)]
                        for m in range(M_T):
                            vslab = bp.tile([128, D], F32R, tag="vslab",
                                            name=f"vslab{b}_{h}_{m}")
                            nc.gpsimd.dma_start(
                                vslab[:], v2.ap()[b, m * 128:(m + 1) * 128, :])
                            for dt in range(KC):
                                nc.tensor.matmul(
                                    psb[dt][:],
                                    vslab[:, dt * 128:(dt + 1) * 128],
                                    t_b[:, m, h * 512:(h + 1) * 512],
                                    start=(m == 0), stop=(m == M_T - 1))
                        for dt in range(KC):
                            nc.vector.tensor_copy(
                                s_sb[b][:, dt, h * 512:(h + 1) * 512], psb[dt][:])

            wp_cm.__exit__(None, None, None)

            # ======== softmax over batch (cross-core) + stage C fused ========
            ap_cm = tc.tile_pool(name="attnp", bufs=1)
            attnp = ap_cm.__enter__()
            attn = [attnp.tile([128, KC, D], BF16, name=f"attn{b}")
                    for b in range(BL)]

            cpool_cm = tc.tile_pool(name="cpool", bufs=3)
            cp_ = cpool_cm.__enter__()
            cps_cm = tc.tile_pool(name="cps", bufs=4, space="PSUM")
            cps = cps_cm.__enter__()

            def load_vT(b, vtp):
                tiles = []
                for m in range(M_T):
                    vb = cp_.tile([128, D], BF16, tag="vb", name=f"vb{b}_{m}")
                    nc.gpsimd.dma_start(
                        vb[:], v2.ap()[b, m * 128:(m + 1) * 128, :])
                    vt = vtp.tile([128, KC, 128], BF16, tag=f"vt{m}",
                                  name=f"vt{b}_{m}")
                    nc.sync.dma_start(vt[:], vb[:], transpose=True)
                    tiles.append(vt)
                return tiles

            def stage_c(b, h, vT):
                for m in range(M_T):
                    ps = cps.tile([128, 512], F32, tag="cps",
                                  name=f"cps{b}_{h}_{m}")
                    for kc in range(KC):
                        nc.tensor.matmul(
                            ps[:], vT[m][:, kc, :],
                            attn[b][:, kc, h * 512:(h + 1) * 512],
                            start=(kc == 0), stop=(kc == KC - 1))
                    ost = cp_.tile([128, 512], F32, tag="ost",
                                   name=f"ost{b}_{h}_{m}")
                    nc.vector.tensor_copy(ost[:], ps[:])
                    nc.sync.dma_start(
                        out2.ap()[b, m * 128:(m + 1) * 128,
                                  h * 512:(h + 1) * 512], ost[:])

            vt0_cm = tc.tile_pool(name="vtp0", bufs=1)
            vtp0 = vt0_cm.__enter__()
            vT0 = load_vT(0, vtp0)  # gpsimd casts drain during B tail; xbars during first AR
            vt1_cm = tc.tile_pool(name="vtp1", bufs=1)
            vtp1 = vt1_cm.__enter__()
            vT1 = load_vT(1, vtp1)

            prev_ar = ar_w2
            with (
                tc.tile_pool(name="sm", bufs=2) as smp,
                tc.tile_pool(name="sm1", bufs=1) as smp1,
            ):
                for h in range(EH):
                    he = slice(h * 512, (h + 1) * 512)
                    # local max per dt-pair chunk -> bf16 bounce (scalar queue)
                    for c in range(ARC):
                        dsl = slice(2 * c, 2 * c + 2)
                        mx = smp.tile([128, 2, 512], BF16, tag="mx",
                                      name=f"mx{h}_{c}")
                        nc.vector.tensor_max(mx[:], s_sb[0][:, dsl, he],
                                             s_sb[1][:, dsl, he])
                        nc.scalar.dma_start(mx_in[h].ap()[:, dsl, :], mx[:])
                    ar_mx = nc.gpsimd.collective_compute(
                        "AllReduce", mybir.AluOpType.max, replica_groups=RG,
                        ins=[mx_in[h].ap().opt()], outs=[mx_out[h].ap().opt()])
                    tile.add_dep_helper(ar_mx.ins, prev_ar.ins, sync=False,
                                        reason="serialize collectives")
                    # subtract gmax, exp, local sum -> f32 bounce
                    for c in range(ARC):
                        dsl = slice(2 * c, 2 * c + 2)
                        gmxb = smp.tile([128, 2, 512], BF16, tag="gmxb",
                                        name=f"gmxb{h}_{c}")
                        nc.scalar.dma_start(gmxb[:], mx_out[h].ap()[:, dsl, :])
                        gmx = smp1.tile([128, 2, 512], F32, tag="gmx",
                                       name=f"gmx{h}_{c}")
                        nc.vector.tensor_copy(gmx[:], gmxb[:])
                        for b in range(BL):
                            nc.vector.tensor_sub(s_sb[b][:, dsl, he],
                                                 s_sb[b][:, dsl, he], gmx[:])
                            nc.scalar.activation(s_sb[b][:, dsl, he],
                                                 s_sb[b][:, dsl, he], AF.Exp)
                        sm = smp1.tile([128, 2, 512], BF16, tag="sm",
                                      name=f"sm{h}_{c}")
                        nc.vector.tensor_add(sm[:], s_sb[0][:, dsl, he],
                                             s_sb[1][:, dsl, he])
                        nc.scalar.dma_start(sm_in[h].ap()[:, dsl, :], sm[:])
                    ar_sm = nc.gpsimd.collective_compute(
                        "AllReduce", mybir.AluOpType.add, replica_groups=RG,
                        ins=[sm_in[h].ap().opt()], outs=[sm_out[h].ap().opt()])
                    tile.add_dep_helper(ar_sm.ins, ar_mx.ins, sync=False,
                                        reason="serialize collectives")
                    prev_ar = ar_sm
                    # 1/Z = exp(-ln(Z)) on ScalarE; attn = p * rec (bf16)
                    for c in range(ARC):
                        dsl = slice(2 * c, 2 * c + 2)
                        zz = smp1.tile([128, 2, 512], BF16, tag="zz",
                                      name=f"zz{h}_{c}")
                        nc.scalar.dma_start(zz[:], sm_out[h].ap()[:, dsl, :])
                        rec = smp1.tile([128, 2, 512], F32, tag="rec",
                                       name=f"rec{h}_{c}")
                        nc.scalar.activation(rec[:], zz[:], AF.Ln)
                        nc.scalar.activation(rec[:], rec[:], AF.Exp, scale=-1.0)
                        for b in range(BL):
                            nc.vector.tensor_mul(attn[b][:, dsl, he],
                                                 s_sb[b][:, dsl, he], rec[:])
                    stage_c(0, h, vT0)
                    stage_c(1, h, vT1)

            vt1_cm.__exit__(None, None, None)
            vt0_cm.__exit__(None, None, None)
            cps_cm.__exit__(None, None, None)
            cpool_cm.__exit__(None, None, None)
            ap_cm.__exit__(None, None, None)
            sp_cm.__exit__(None, None, None)

    nc.compile()
    return nc


_NC = None


def _get_nc():
    global _NC
    if _NC is None:
        _NC = build()
    return _NC


def kernel(q, k, v, W, U):
    q = np.ascontiguousarray(np.asarray(q, dtype=np.float32))
    k = np.ascontiguousarray(np.asarray(k, dtype=np.float32))
    v = np.ascontiguousarray(np.asarray(v, dtype=np.float32))
    W = np.ascontiguousarray(np.asarray(W, dtype=np.float32))
    U = np.ascontiguousarray(np.asarray(U, dtype=np.float32))

    nc = _get_nc()
    in_maps = [
        {
            "q2": q[c * BL:(c + 1) * BL],
            "k2": k[c * BL:(c + 1) * BL],
            "v2": v[c * BL:(c + 1) * BL],
            "W": W,
            "U": U,
        }
        for c in range(N_CORES)
    ]
    res = run_bass_kernel_spmd(nc, in_maps, core_ids=list(range(N_CORES)))
    out = np.concatenate([res.results[c]["out"] for c in range(N_CORES)], axis=0)
    return out.astype(np.float32)


if __name__ == "__main__":
    rng = np.random.default_rng(0)
    q = rng.standard_normal((B, D), dtype=np.float32)
    k = rng.standard_normal((B, S, D), dtype=np.float32)
    v = rng.standard_normal((B, S, D), dtype=np.float32)
    W = (rng.standard_normal((D, D), dtype=np.float32) / np.sqrt(D)).astype(np.float32)
    U = (rng.standard_normal((D, D), dtype=np.float32) / np.sqrt(D)).astype(np.float32)
    out = kernel(q=q, k=k, v=v, W=W, U=U)
    print("out", out.shape, out.dtype, float(np.abs(out).mean()))

